# revision 1
# baseline (speedup 1.0000x reference)
"""Trainium2 Bass kernel for nn_Attention_73701638800162.

Channel attention (XCA-style) with C=3 channels, N=1024*1024 spatial, B=4.
  q  = dw3x3(conv1x1(fhigh, q_C_w), q_dw_w)
  k  = dw3x3(conv1x1(x_planes, kv_C_w), kv_dw_w);  v = k
  attn = softmax(l2norm(q) @ l2norm(k).T * temp)      # [3,3] per batch
  out  = proj_w @ (attn @ k) + proj_b                  # -> [B, N, C]

Key algebra: out = Mmix @ k + b where Mmix = proj_w @ softmax(S/(|q||k|)),
S[c,d] = sum_n q_c k_d. Only 15 global scalars (9 S, 3 |q|^2, 3 |k|^2) are
needed besides k itself, so k stays resident in SBUF between the two passes.

Sharding: 8 cores = 4 batches x 2 H-halves. The 15 partial stats are
all-reduced between the 2 spatial shards of each batch (replica pairs).

Per-core layout: planes [3, 512(+halo), 1024]; 16 row-tiles of R=32 rows,
channel-stacked partitions with 32-row blocks (partition bases must be
multiples of 32): psum/product partitions (c*32 + r), c = 0..2, block 3
dummy-zero. The fused 3x3 conv (1x1 then depthwise) becomes 3 accumulating
PE matmuls (one per kx) with host-precomputed banded weight matrices
[102, 128]; W-shifts are free-dim offsets on zero-padded inputs (W+2).
Stats: DVE channel-rotated products (q from PSUM x k from SBUF, bases all
32-aligned) + PE selector-matmul reductions into persistent PSUM tiles.
"""
import sys
if '/opt/trn_rl_repo' not in sys.path:
    sys.path.insert(0, '/opt/trn_rl_repo')

import numpy as np

B, H, W, C = 4, 1024, 1024, 3
N = H * W
HH = H // 2                 # rows per core-shard (512)
R = 32                      # output rows per tile position
NPOS = HH // R              # 16 positions, uniform
WP = W + 2                  # zero-padded width
M = 128                     # psum partitions: 4 blocks of 32 (block 3 dummy)
KIN = R + 2                 # input rows per channel (34)
KF = 3 * KIN                # contraction dim (102)

_PROGRAM = None


def _band_matrix(Wfull):
    """Conv lhsT [102, 3*128] (kx-major): col (c*32+r), row (d*34+rp);
    value Wfull[c,d,rp-r,kx]. Dummy block columns 96..127 stay zero."""
    mat = np.zeros((KF, 3, M), dtype=np.float32)
    for kx in range(3):
        for d in range(3):
            for c in range(3):
                for r in range(R):
                    for ky in range(3):
                        mat[d * KIN + r + ky, kx, c * R + r] = Wfull[c, d, ky, kx]
    return mat.reshape(KF, 3 * M)


def _selectors():
    """Selector lhsT matrices mapping product-tile partitions (32-blocks)
    to stat rows. g[X] col 3X+c selects block c; gq/gk for squares."""
    g = [np.zeros((M, 9), np.float32) for _ in range(3)]
    for X in range(3):
        for c in range(3):
            g[X][c * R:(c + 1) * R, 3 * X + c] = 1.0
    gq = np.zeros((M, 6), np.float32)
    gk = np.zeros((M, 6), np.float32)
    for c in range(3):
        gq[c * R:(c + 1) * R, c] = 1.0
        gk[c * R:(c + 1) * R, 3 + c] = 1.0
    return g[0], g[1], g[2], gq, gk


def _emasks():
    """E_j [128, 128], j = 3*cp + d: E[d*32+r, cp*32+r] = 1. Concat along
    columns -> [128, 9*128]. Dummy blocks stay zero."""
    E = np.zeros((9, M, M), np.float32)
    for cp in range(3):
        for d in range(3):
            j = 3 * cp + d
            for r in range(R):
                E[j, d * R + r, cp * R + r] = 1.0
    return E.transpose(1, 0, 2).reshape(M, 9 * M)


def _build_program(temp, stage=5, npos=NPOS):
    import concourse.bass as bass  # noqa: F401
    import concourse.bacc as bacc
    import concourse.mybir as mybir
    import concourse.tile as tile

    DT = mybir.dt.float32
    F32R = mybir.dt.float32r
    BF16 = mybir.dt.bfloat16
    AL = mybir.AluOpType
    AF = mybir.ActivationFunctionType

    nc = bacc.Bacc("TRN2", target_bir_lowering=False, debug=False, num_devices=8)

    fh_e = nc.declare_dram_parameter("fh", [3, HH + 2, WP], DT, isOutput=False)
    xs_e = nc.declare_dram_parameter("xs", [3, HH + 2, WP], DT, isOutput=False)
    mq_e = nc.declare_dram_parameter("mq", [KF, 3 * M], DT, isOutput=False)
    mk_e = nc.declare_dram_parameter("mk", [KF, 3 * M], DT, isOutput=False)
    g_e = nc.declare_dram_parameter("gsel", [M, 27], DT, isOutput=False)
    gsq_e = nc.declare_dram_parameter("gsq", [M, 12], DT, isOutput=False)
    em_e = nc.declare_dram_parameter("emask", [M, 9 * M], DT, isOutput=False)
    pj_e = nc.declare_dram_parameter("projc", [1, 9], DT, isOutput=False)
    bc_e = nc.declare_dram_parameter("bcol", [M, 1], DT, isOutput=False)
    pm_e = nc.declare_dram_parameter("perm", [M, 2 * M], DT, isOutput=False)
    out_e = nc.declare_dram_parameter("out", [3, HH, W], DT, isOutput=True)

    with tile.TileContext(nc) as tc:
        with tc.tile_pool(name="const", bufs=1) as cst, \
             tc.tile_pool(name="ksto", bufs=1) as kst, \
             tc.tile_pool(name="io", bufs=3) as io, \
             tc.tile_pool(name="work", bufs=3) as wk_p, \
             tc.tile_pool(name="small", bufs=1) as sm, \
             tc.tile_pool(name="dram", bufs=1, space="DRAM") as dr:

            # ---- constants into SBUF
            mq_t = cst.tile([KF, 3 * M], F32R, tag="mq")
            mk_t = cst.tile([KF, 3 * M], F32R, tag="mk")
            g_t = cst.tile([M, 27], BF16, tag="gsel")
            gsq_t = cst.tile([M, 12], BF16, tag="gsq")
            em_t = cst.tile([M, 9 * M], BF16, tag="emask")
            pj_t = cst.tile([1, 9], DT, tag="projc")
            bc_t = cst.tile([M, 1], DT, tag="bcol")
            nc.sync.dma_start(mq_t[:], mq_e[:].bitcast(F32R))
            nc.sync.dma_start(mk_t[:], mk_e[:].bitcast(F32R))
            nc.sync.dma_start(pj_t[:], pj_e[:])
            nc.sync.dma_start(bc_t[:], bc_e[:])
            # f32 -> bf16 via ACT copies (DMA can't convert)
            gf = sm.tile([M, 27], DT, tag="gf")
            nc.sync.dma_start(gf[:], g_e[:])
            nc.scalar.copy(out=g_t[:], in_=gf[:])
            gsf = sm.tile([M, 12], DT, tag="gsf")
            nc.sync.dma_start(gsf[:], gsq_e[:])
            nc.scalar.copy(out=gsq_t[:], in_=gsf[:])
            emf = sm.tile([M, 9 * M], DT, tag="emf")
            nc.sync.dma_start(emf[:], em_e[:])
            nc.scalar.copy(out=em_t[:], in_=emf[:])
            pm_t = cst.tile([M, 2 * M], BF16, tag="perm")
            pmf = sm.tile([M, 2 * M], DT, tag="pmf")
            nc.sync.dma_start(pmf[:], pm_e[:])
            nc.scalar.copy(out=pm_t[:], in_=pmf[:])

            ks = [kst.tile([M, W], BF16, tag=f"k{p}", name=f"k{p}")
                  for p in range(NPOS)]

            # ================= phase 1: convs + stats =================
            with tc.tile_pool(name="pq", bufs=2, space="PSUM") as pqp, \
                 tc.tile_pool(name="pk", bufs=2, space="PSUM") as pkp, \
                 tc.tile_pool(name="pkr", bufs=1, space="PSUM") as pkr, \
                 tc.tile_pool(name="pstat", bufs=1, space="PSUM") as pst, \
                 tc.tile_pool(name="psq", bufs=1, space="PSUM") as psq:

                stat_t = pst.tile([9, 512], DT, tag="stat")
                sqs_t = psq.tile([6, 512], DT, tag="sqs")

                for p in range(npos):
                    inq = io.tile([KF, WP], F32R, tag="inq")
                    ink = io.tile([KF, WP], F32R, tag="ink")
                    for c in range(3):
                        nc.sync.dma_start(
                            inq[c * KIN:(c + 1) * KIN, :],
                            fh_e[c, R * p:R * p + KIN, :].bitcast(F32R))
                        nc.sync.dma_start(
                            ink[c * KIN:(c + 1) * KIN, :],
                            xs_e[c, R * p:R * p + KIN, :].bitcast(F32R))
                    for h in range(2):
                        pq_t = pqp.tile([M, 512], DT, tag="pq")
                        pk_t = pkp.tile([M, 512], DT, tag="pk")
                        for kx in range(3):
                            nc.tensor.matmul(
                                pq_t[:], mq_t[:, M * kx:M * (kx + 1)],
                                inq[:, kx + 512 * h: kx + 512 * h + 512],
                                start=(kx == 0), stop=(kx == 2))
                        for kx in range(3):
                            nc.tensor.matmul(
                                pk_t[:], mk_t[:, M * kx:M * (kx + 1)],
                                ink[:, kx + 512 * h: kx + 512 * h + 512],
                                start=(kx == 0), stop=(kx == 2))
                        sl = slice(512 * h, 512 * (h + 1))
                        nc.scalar.copy(out=ks[p][:, sl], in_=pk_t[:])
                        qs = wk_p.tile([M, 512], BF16, tag="qs")
                        nc.scalar.copy(out=qs[:], in_=pq_t[:])
                        # rotated k replicas via PE permutation matmuls
                        kr1 = pkr.tile([M, 512], DT, tag="kr", name="kr1")
                        kr2 = pkr.tile([M, 512], DT, tag="kr", name="kr2")
                        nc.tensor.matmul(kr1[:], pm_t[:, 0:M], ks[p][:, sl],
                                         start=True, stop=True)
                        nc.tensor.matmul(kr2[:], pm_t[:, M:2 * M], ks[p][:, sl],
                                         start=True, stop=True)
                        # full-width products, all base 0
                        pr0 = wk_p.tile([M, 512], BF16, tag="pr0")
                        pr1 = wk_p.tile([M, 512], BF16, tag="pr1")
                        pr2 = wk_p.tile([M, 512], BF16, tag="pr2")
                        nc.vector.tensor_tensor(
                            out=pr0[:], in0=qs[:], in1=ks[p][:, sl], op=AL.mult)
                        nc.vector.tensor_tensor(
                            out=pr1[:], in0=kr1[:], in1=qs[:], op=AL.mult)
                        nc.vector.tensor_tensor(
                            out=pr2[:], in0=kr2[:], in1=qs[:], op=AL.mult)
                        # squares on Pool (SBUF-only engine)
                        sq_q = wk_p.tile([M, 512], BF16, tag="sqq")
                        sq_k = wk_p.tile([M, 512], BF16, tag="sqk")
                        nc.gpsimd.tensor_tensor(
                            out=sq_q[:], in0=qs[:], in1=qs[:], op=AL.mult)
                        nc.gpsimd.tensor_tensor(
                            out=sq_k[:], in0=ks[p][:, sl], in1=ks[p][:, sl],
                            op=AL.mult)
                        first = (p == 0 and h == 0)
                        last = (p == npos - 1 and h == 1)
                        for X, pr in enumerate((pr0, pr1, pr2)):
                            nc.tensor.matmul(
                                stat_t[:], g_t[:, 9 * X:9 * (X + 1)], pr[:],
                                start=(first and X == 0), stop=(last and X == 2),
                                skip_group_check=True)
                        nc.tensor.matmul(
                            sqs_t[:], gsq_t[:, 0:6], sq_q[:],
                            start=first, stop=False, skip_group_check=True)
                        nc.tensor.matmul(
                            sqs_t[:], gsq_t[:, 6:12], sq_k[:],
                            start=False, stop=last, skip_group_check=True)
                        if stage == 0 and p == 0 and h == 0:
                            for di, dt_ in enumerate(
                                    (qs, ks[p][:, sl], kr1, kr2,
                                     pr0, pr1, pr2, sq_q)):
                                dbg = io.tile([M, 512], DT, tag="obuf",
                                              name=f"dbg{di}")
                                nc.scalar.copy(out=dbg[:], in_=dt_)
                                nc.sync.dma_start(
                                    out_e[di // 4,
                                          (di % 4) * M:(di % 4) * M + M,
                                          0:512],
                                    dbg[:])

                statcol = sm.tile([9, 1], DT, tag="statcol")
                sqcol = sm.tile([6, 1], DT, tag="sqcol")
                nc.vector.tensor_reduce(
                    out=statcol[:], in_=stat_t[:], axis=mybir.AxisListType.X,
                    op=AL.add)
                nc.vector.tensor_reduce(
                    out=sqcol[:], in_=sqs_t[:], axis=mybir.AxisListType.X,
                    op=AL.add)
                if stage < 2:
                    # second opinion: SBUF copy then reduce
                    scpy = sm.tile([9, 512], DT, tag="scpy")
                    nc.vector.tensor_copy(scpy[:], stat_t[:])
                    statcol2 = sm.tile([9, 1], DT, tag="statcol2")
                    nc.vector.tensor_reduce(
                        out=statcol2[:], in_=scpy[:],
                        axis=mybir.AxisListType.X, op=AL.add)
                    nc.sync.dma_start(out_e[2, 0:9, 0:512], scpy[:])
                statcol = statcol[:]
                sqcol = sqcol[:]

            # ================= all-reduce the 15 scalars =================
            if stage < 2:
                nc.sync.dma_start(out_e[0, 0, 0:9], statcol.rearrange("a b -> b a"))
                nc.sync.dma_start(out_e[0, 1, 0:6], sqcol.rearrange("a b -> b a"))
                nc.sync.dma_start(out_e[0, 2, 0:9], statcol2[:].rearrange("a b -> b a"))
                sdump = io.tile([9, 512], DT, tag="obuf", name="sdump")
                nc.scalar.copy(out=sdump[:], in_=stat_t[:])
                nc.sync.dma_start(out_e[1, 0:9, 0:512], sdump[:])
                qdump = io.tile([6, 512], DT, tag="obuf", name="qdump")
                nc.scalar.copy(out=qdump[:], in_=sqs_t[:])
                nc.sync.dma_start(out_e[1, 16:22, 0:512], qdump[:])
            if stage >= 2:
                arin = dr.tile([15, 1], DT, tag="arin")
                arout = dr.tile([15, 1], DT, tag="arout")
                nc.sync.dma_start(arin[0:9, :], statcol)
                nc.sync.dma_start(arin[9:15, :], sqcol)
                nc.gpsimd.collective_compute(
                    "AllReduce", AL.add,
                    replica_groups=[[0, 1], [2, 3], [4, 5], [6, 7]],
                    ins=[arin[:].opt()], outs=[arout[:].opt()])
                srow = sm.tile([1, 15], DT, tag="srow")
                nc.sync.dma_start(srow[:], arout[:].rearrange("a b -> b a"))
                if stage == 2:
                    nc.sync.dma_start(out_e[0, 0, 0:15], srow[:])

            if stage >= 3:
                # ================= tiny softmax / Mmix =================
                # srow layout: [0:9] S in (X, c) X-major; [9:12] |q|^2; [12:15] |k|^2
                rts = sm.tile([1, 6], DT, tag="rts")       # |q|, |k|
                nc.scalar.activation(out=rts[:], in_=srow[:, 9:15], func=AF.Sqrt)
                rcp = sm.tile([1, 6], DT, tag="rcp")       # 1/|q|, 1/|k|
                nc.vector.reciprocal(out=rcp[:], in_=rts[:])
                # rk9[(c,X)] = 1/|k|[(c+X)%3], c-major, via 3 strided copies
                rk9 = sm.tile([1, 9], DT, tag="rk9")
                rkd = sm.tile([1, 6], DT, tag="rkd")       # 1/|k| duplicated x2
                nc.vector.tensor_copy(rkd[:, 0:3], rcp[:, 3:6])
                nc.vector.tensor_copy(rkd[:, 3:6], rcp[:, 3:6])
                for X in range(3):
                    nc.vector.tensor_copy(
                        rk9[:].rearrange("a (c x) -> a c x", c=3)[:, :, X],
                        rkd[:, X:X + 3])
                # logits (c, X) c-major: S[(c,X)] * (1/|q|)[c] * rk9
                sv = srow[:, 0:9].rearrange("a (x c) -> a c x", x=3)      # (c,X) view
                lg = sm.tile([1, 9], DT, tag="lg")
                lgv = lg[:].rearrange("a (c x) -> a c x", c=3)
                rqb = rcp[:, 0:3].unsqueeze(2).broadcast_to((1, 3, 3))
                nc.vector.tensor_tensor(out=lgv, in0=sv, in1=rqb, op=AL.mult)
                nc.vector.tensor_tensor(out=lg[:], in0=lg[:], in1=rk9[:], op=AL.mult)
                ex = sm.tile([1, 9], DT, tag="ex")
                nc.scalar.activation(out=ex[:], in_=lg[:], func=AF.Exp, scale=temp)
                se = sm.tile([1, 3], DT, tag="se")
                nc.vector.tensor_reduce(
                    out=se[:].unsqueeze(2),
                    in_=ex[:].rearrange("a (c x) -> a c x", c=3),
                    axis=mybir.AxisListType.X, op=AL.add)
                rse = sm.tile([1, 3], DT, tag="rse")
                nc.vector.reciprocal(out=rse[:], in_=se[:])
                at = sm.tile([1, 9], DT, tag="at")          # attn (c, X) c-major
                nc.vector.tensor_tensor(
                    out=at[:].rearrange("a (c x) -> a c x", c=3),
                    in0=ex[:].rearrange("a (c x) -> a c x", c=3),
                    in1=rse[:].unsqueeze(2).broadcast_to((1, 3, 3)), op=AL.mult)
                # attndup [1, 18]: row a holds [attn(a,0..2) attn(a,0..2)]
                ad = sm.tile([1, 18], DT, tag="ad")
                adv = ad[:].rearrange("a (g s) -> a g s", g=3)
                atv = at[:].rearrange("a (c x) -> a c x", c=3)
                nc.vector.tensor_copy(adv[:, :, 0:3], atv)
                nc.vector.tensor_copy(adv[:, :, 3:6], atv)
                # m9[cp, d] = sum_a proj[cp, a] * attn[a, (d - a) % 3]
                m9 = sm.tile([1, 9], DT, tag="m9")
                tmp9 = sm.tile([1, 9], DT, tag="tmp9")
                for a in range(3):
                    off = 6 * a + ((3 - a) % 3)
                    att_a = ad[:, off:off + 3].unsqueeze(1).broadcast_to((1, 3, 3))
                    pj_a = pj_t[:, 3 * a:3 * a + 3].unsqueeze(2).broadcast_to((1, 3, 3))
                    dst = m9 if a == 0 else tmp9
                    nc.vector.tensor_tensor(
                        out=dst[:].rearrange("a (c d) -> a c d", c=3),
                        in0=pj_a, in1=att_a, op=AL.mult)
                    if a > 0:
                        nc.vector.tensor_tensor(
                            out=m9[:], in0=m9[:], in1=tmp9[:], op=AL.add)
                if stage == 3:
                    nc.sync.dma_start(out_e[0, 0, 0:9], m9[:])

            if stage >= 4:
                # broadcast m9 down partitions, build banded mix lhsT [128, 128]
                mcols = sm.tile([M, 9], DT, tag="mcols")
                nc.gpsimd.partition_broadcast(mcols[:], m9[:])
                mixw = sm.tile([M, M], BF16, tag="mixw")
                mtmp = sm.tile([M, M], BF16, tag="mtmp")
                for j in range(9):
                    dst = mixw if j == 0 else mtmp
                    nc.vector.tensor_scalar_mul(
                        out=dst[:], in0=em_t[:, M * j:M * (j + 1)],
                        scalar1=mcols[:, j:j + 1])
                    if j > 0:
                        nc.vector.tensor_tensor(
                            out=mixw[:], in0=mixw[:], in1=mtmp[:], op=AL.add)
                if stage == 4:
                    ob0 = io.tile([M, M], DT, tag="obuf", name="ob0")
                    nc.scalar.copy(out=ob0[:], in_=mixw[:])
                    nc.sync.dma_start(out_e[0, 0:128, 0:128], ob0[:])

            if stage >= 5:
                # ================= phase 2: out = mixw @ k + b =================
                with tc.tile_pool(name="pmix", bufs=4, space="PSUM") as pmx:
                    for p in range(NPOS):
                        ob = io.tile([M, W], DT, tag="obuf")
                        for h in range(2):
                            po = pmx.tile([M, 512], DT, tag="po")
                            nc.tensor.matmul(
                                po[:], mixw[:], ks[p][:, 512 * h:512 * (h + 1)],
                                start=True, stop=True)
                            nc.scalar.activation(
                                out=ob[:, 512 * h:512 * (h + 1)], in_=po[:],
                                func=AF.Identity, bias=bc_t[:, 0:1])
                        for c in range(3):
                            nc.sync.dma_start(
                                out_e[c, R * p:R * p + R, :],
                                ob[R * c:R * c + R, :])

    nc.finalize()
    return nc


def _prep_in_maps(x, fhigh, q_C_w, q_dw_w, kv_C_w, kv_dw_w, proj_w, proj_b):
    """Host-side shard/layout prep shared by kernel() and test profiling."""
    wq = q_dw_w[:, 0, :, :][:, None] * q_C_w[:, :, 0, 0][:, :, None, None]
    wk = kv_dw_w[:, 0, :, :][:, None] * kv_C_w[:, :, 0, 0][:, :, None, None]
    mq = _band_matrix(wq)
    mk = _band_matrix(wk)
    g0, g1, g2, gsqq, gsqk = _selectors()
    gsel = np.concatenate([g0, g1, g2], axis=1)
    gsq = np.concatenate([gsqq, gsqk], axis=1)
    emask = _emasks()
    projc = proj_w[:, :, 0, 0].T.reshape(1, 9).copy()   # (a, c') a-major
    perm = np.zeros((M, 2 * M), np.float32)
    for X in (1, 2):
        for c in range(3):
            for r in range(R):
                # kr_X[(c,r)] = k[((c+X)%3, r)]
                perm[((c + X) % 3) * R + r, (X - 1) * M + c * R + r] = 1.0
    bcol = np.zeros((M, 1), np.float32)
    bcol[0:96, 0] = np.repeat(proj_b, R)

    fhp = np.pad(fhigh, ((0, 0), (0, 0), (1, 1), (1, 1)))
    xpl = np.ascontiguousarray(x.transpose(0, 2, 1)).reshape(B, 3, H, W)
    xpp = np.pad(xpl, ((0, 0), (0, 0), (1, 1), (1, 1)))

    shared = dict(mq=mq, mk=mk, gsel=gsel, gsq=gsq, emask=emask,
                  projc=projc, bcol=bcol, perm=perm)
    in_maps = []
    for core in range(8):
        b, half = core // 2, core % 2
        s = half * HH
        m = dict(shared)
        m["fh"] = np.ascontiguousarray(fhp[b][:, s:s + HH + 2, :])
        m["xs"] = np.ascontiguousarray(xpp[b][:, s:s + HH + 2, :])
        in_maps.append(m)
    return in_maps


def kernel(x, fhigh, q_C_w, q_dw_w, kv_C_w, kv_dw_w, proj_w, proj_b,
           temperature):
    from concourse.bass_utils import run_bass_kernel_spmd

    x = np.asarray(x, dtype=np.float32)
    fhigh = np.asarray(fhigh, dtype=np.float32)
    args = [np.asarray(a, dtype=np.float32) for a in
            (q_C_w, q_dw_w, kv_C_w, kv_dw_w, proj_w, proj_b)]
    temp = float(np.asarray(temperature).reshape(-1)[0])

    global _PROGRAM
    if _PROGRAM is None:
        _PROGRAM = _build_program(temp)
    in_maps = _prep_in_maps(x, fhigh, *args)
    res = run_bass_kernel_spmd(_PROGRAM, in_maps, core_ids=list(range(8)))

    out = np.empty((B, N, C), dtype=np.float32)
    for core in range(8):
        b, half = core // 2, core % 2
        planes = res.results[core]["out"]          # [3, 512, 1024]
        flat = planes.reshape(3, HH * W).T         # [HH*W, 3]
        out[b, half * HH * W:(half + 1) * HH * W, :] = flat
    return out

